# revision 1
# baseline (speedup 1.0000x reference)
"""Trainium2 Bass kernel for nn_KDE: log_p[b] = logsumexp_n(-scale*||X_b - svs_n||^2)
                                               - log(N) + (D/2)*log(scale/pi)

Strategy (8 NeuronCores, SPMD):
  - svs sharded along N: each core owns 8192 support vectors; X replicated.
  - Per core, on device:
      * build augmented matrices  xt_aug  = [[2*s*X^T], [1...1]]      (bf16, [65, 2048])
                                  svst_aug = [[svs^T], [-s*||y||^2]]  (bf16, [65, 8192])
        (the -s*||y||^2 row is computed on device from svs^T via DVE square +
         ones-vector matmul on the PE)
      * one bf16 matmul per [128 query, 512 sv] tile yields the exp argument
          a[b, n] = 2*s*x_b.y_n - s*||y_n||^2   accumulated fp32 in PSUM
      * ScalarE (ACT) applies Exp over [128, 2048] PSUM tiles (4 banks), DVE
        reduces each exp tile along the sv axis -> per-query partial sums
      * device also emits xrow[b] = -s*||x_b||^2 - log(N) + (D/2)*log(s/pi)
  - Host combine (the cross-device logsumexp step, shards are disjoint):
      out = log(sum_cores partial) + xrow
"""

import sys
from contextlib import ExitStack

import numpy as np


def _ensure_concourse():
    try:
        import concourse  # noqa: F401
    except ImportError:
        sys.path.insert(0, "/opt/trn_rl_repo")


_ensure_concourse()

import ml_dtypes  # noqa: E402

import concourse.bacc as bacc  # noqa: E402
import concourse.tile as tile  # noqa: E402
from concourse import mybir  # noqa: E402
from concourse.bass_utils import run_bass_kernel_spmd  # noqa: E402

N_CORES = 8
B = 2048          # queries
N_TOTAL = 65536   # support vectors
D = 64            # feature dim
NSH = N_TOTAL // N_CORES  # 8192 svs per core

BT = 128      # query tile (PSUM partitions)
NB = 512      # matmul moving free dim (one fp32 PSUM bank)
GROUP = 2048  # ACT call free size (4 PSUM banks)
N_MCHUNK = B // BT        # 16
N_GROUP = NSH // GROUP    # 4
JPG = GROUP // NB         # 4 matmuls per group

F32 = mybir.dt.float32
BF16 = mybir.dt.bfloat16

_PROGRAM_CACHE: dict[float, object] = {}
LAST_RESULTS = None  # BassKernelResults of the most recent run (for profiling)


def _build_program(s: float):
    AF = mybir.ActivationFunctionType
    ALU = mybir.AluOpType
    AX = mybir.AxisListType

    nc = bacc.Bacc(
        "TRN2",
        target_bir_lowering=False,
        debug=False,
        enable_asserts=False,
        num_devices=N_CORES,
    )
    svsT_d = nc.dram_tensor("svsT", [D, NSH], BF16, kind="ExternalInput").ap()
    xT_d = nc.dram_tensor("xT", [D, B], F32, kind="ExternalInput").ap()
    partial_d = nc.dram_tensor("partial", [B], F32, kind="ExternalOutput").ap()
    xrow_d = nc.dram_tensor("xrow", [B], F32, kind="ExternalOutput").ap()

    # constant folded into the per-query row (uses the GLOBAL N)
    cconst = float(-np.log(N_TOTAL) + (D / 2.0) * np.log(s / np.pi))

    with tile.TileContext(nc) as tc, ExitStack() as ctx:
        aug = ctx.enter_context(tc.tile_pool(name="aug", bufs=1))
        pp = ctx.enter_context(tc.tile_pool(name="psum", bufs=2, space="PSUM"))
        sp = ctx.enter_context(tc.tile_pool(name="scr", bufs=2))
        misc = ctx.enter_context(tc.tile_pool(name="misc", bufs=1))
        rowp = ctx.enter_context(tc.tile_pool(name="rowp", bufs=2))

        svst_aug = aug.tile([D + 1, NSH], BF16)
        xt_aug = aug.tile([D + 1, B], BF16)
        sq = misc.tile([D, NSH], BF16)       # svs^T squared elementwise
        xts = misc.tile([D, B], F32)         # raw X^T
        sqx = misc.tile([D, B], BF16)        # X^T squared elementwise
        ones = misc.tile([D, 1], BF16)
        accall = misc.tile([BT, N_MCHUNK * N_GROUP], F32)
        outp = misc.tile([BT, N_MCHUNK], F32)
        xrow_sb = misc.tile([1, B], F32)

        nc.vector.memset(ones[:, :], 1.0)

        # ---- input DMAs (chunked for overlap) ----
        for k in range(8):
            c0 = k * (NSH // 8)
            c1 = c0 + NSH // 8
            nc.sync.dma_start(out=svst_aug[0:D, c0:c1], in_=svsT_d[:, c0:c1])
        for k in range(2):
            c0 = k * (B // 2)
            c1 = c0 + B // 2
            nc.sync.dma_start(out=xts[:, c0:c1], in_=xT_d[:, c0:c1])

        # ---- X-side prep: xt_aug = [2s*X^T ; ones], sqx = (X^T)^2 ----
        nc.vector.tensor_scalar_mul(xt_aug[0:D, :], xts[:, :], 2.0 * s)
        nc.vector.memset(xt_aug[D : D + 1, :], 1.0)
        nc.vector.tensor_mul(sqx[:, :], xts[:, :], xts[:, :])

        # ---- xrow = -s*||x||^2 + cconst  (ones-matmul partition reduction) ----
        psx = pp.tile([BT, GROUP], F32, tag="mm")
        for j in range(JPG):
            nc.tensor.matmul(
                psx[0:1, j * NB : (j + 1) * NB],
                lhsT=ones[:, :],
                rhs=sqx[:, j * NB : (j + 1) * NB],
                start=True,
                stop=True,
            )
        nc.vector.tensor_scalar(
            xrow_sb[0:1, :], psx[0:1, 0:B], -s, cconst, op0=ALU.mult, op1=ALU.add
        )
        nc.sync.dma_start(out=xrow_d[:], in_=xrow_sb[0:1, :])

        # ---- y2 row: svst_aug[64, :] = -s * ||y||^2 ----
        for g in range(N_GROUP):
            gc0 = g * GROUP
            nc.vector.tensor_mul(
                sq[:, gc0 : gc0 + GROUP],
                svst_aug[0:D, gc0 : gc0 + GROUP],
                svst_aug[0:D, gc0 : gc0 + GROUP],
            )
            psy = pp.tile([BT, GROUP], F32, tag="mm")
            for j in range(JPG):
                c0 = gc0 + j * NB
                nc.tensor.matmul(
                    psy[0:1, j * NB : (j + 1) * NB],
                    lhsT=ones[:, :],
                    rhs=sq[:, c0 : c0 + NB],
                    start=True,
                    stop=True,
                )
            yrow = rowp.tile([1, GROUP], BF16)
            nc.vector.tensor_scalar_mul(yrow[0:1, :], psy[0:1, :], -s)
            # move row from partition 0 to partition 64 (SBUF->SBUF DMA)
            nc.sync.dma_start(
                out=svst_aug[D : D + 1, gc0 : gc0 + GROUP], in_=yrow[0:1, :]
            )

        # ---- main loop: matmul -> exp -> reduce ----
        for m in range(N_MCHUNK):
            for g in range(N_GROUP):
                idx = m * N_GROUP + g
                ps = pp.tile([BT, GROUP], F32, tag="mm")
                for j in range(JPG):
                    col = g * GROUP + j * NB
                    nc.tensor.matmul(
                        ps[:, j * NB : (j + 1) * NB],
                        lhsT=xt_aug[:, m * BT : (m + 1) * BT],
                        rhs=svst_aug[:, col : col + NB],
                        start=True,
                        stop=True,
                    )
                scr = sp.tile([BT, GROUP], BF16)
                nc.scalar.activation(scr[:, :], ps[:, :], AF.Exp)
                nc.vector.tensor_reduce(
                    accall[:, idx : idx + 1], scr[:, :], axis=AX.X, op=ALU.add
                )

        # ---- fold the per-group partials and store ----
        acc3 = accall[:, :].rearrange("p (m g) -> p m g", g=N_GROUP)
        nc.vector.tensor_reduce(outp[:, :], acc3, axis=AX.X, op=ALU.add)
        nc.sync.dma_start(
            out=partial_d.rearrange("(m p) -> p m", p=BT), in_=outp[:, :]
        )

    nc.compile()
    return nc


def _get_program(s: float):
    key = float(s)
    if key not in _PROGRAM_CACHE:
        _PROGRAM_CACHE[key] = _build_program(key)
    return _PROGRAM_CACHE[key]


def kernel(X, svs, scale, _trace=False):
    global LAST_RESULTS
    Xnp = np.asarray(X, dtype=np.float32)
    svs_np = np.asarray(svs, dtype=np.float32)
    s = float(np.asarray(scale))
    assert Xnp.shape == (B, D) and svs_np.shape == (N_TOTAL, D)

    nc = _get_program(s)

    xT = np.ascontiguousarray(Xnp.T)  # [64, 2048] f32, replicated
    in_maps = []
    for c in range(N_CORES):
        shard = svs_np[c * NSH : (c + 1) * NSH]
        svsT_c = np.ascontiguousarray(shard.T).astype(ml_dtypes.bfloat16)
        in_maps.append({"svsT": svsT_c, "xT": xT})

    res = run_bass_kernel_spmd(nc, in_maps, list(range(N_CORES)), trace=_trace)
    LAST_RESULTS = res

    partials = np.stack(
        [np.asarray(r["partial"], dtype=np.float64) for r in res.results]
    )  # [8, 2048]
    xrow = np.asarray(res.results[0]["xrow"], dtype=np.float64)
    out = np.log(partials.sum(axis=0)) + xrow
    return out.astype(np.float32)


# revision 6
# speedup vs baseline: 1.0316x; 1.0316x over previous
"""Trainium2 Bass kernel for nn_KDE: log_p[b] = logsumexp_n(-scale*||X_b - svs_n||^2)
                                               - log(N) + (D/2)*log(scale/pi)

Strategy (8 NeuronCores, SPMD):
  - svs sharded along N: each core owns 8192 support vectors; X replicated.
  - Per core, on device:
      * build augmented matrices  xt_aug  = [[2*s*X^T], [1...1]]      (bf16, [65, 2048])
                                  svst_aug = [[svs^T], [-s*||y||^2]]  (bf16, [65, 8192])
        (the -s*||y||^2 row is computed on device from svs^T via DVE square +
         ones-vector matmul on the PE)
      * one bf16 matmul per [128 query, 512 sv] tile yields the exp argument
          a[b, n] = 2*s*x_b.y_n - s*||y_n||^2   accumulated fp32 in PSUM
      * ScalarE (ACT) applies Exp over [128, 2048] PSUM tiles (4 banks), DVE
        reduces each exp tile along the sv axis -> per-query partial sums
      * device also emits xrow[b] = -s*||x_b||^2 - log(N) + (D/2)*log(s/pi)
  - Host combine (the cross-device logsumexp step, shards are disjoint):
      out = log(sum_cores partial) + xrow
"""

import sys
from contextlib import ExitStack

import numpy as np


def _ensure_concourse():
    try:
        import concourse  # noqa: F401
    except ImportError:
        sys.path.insert(0, "/opt/trn_rl_repo")


_ensure_concourse()

import ml_dtypes  # noqa: E402

import concourse.bacc as bacc  # noqa: E402
import concourse.tile as tile  # noqa: E402
from concourse import mybir  # noqa: E402
from concourse.bass_utils import run_bass_kernel_spmd  # noqa: E402

N_CORES = 8
B = 2048          # queries
N_TOTAL = 65536   # support vectors
D = 64            # feature dim
NSH = N_TOTAL // N_CORES  # 8192 svs per core

BT = 128      # query tile (PSUM partitions)
NB = 512      # matmul moving free dim (one fp32 PSUM bank)
GROUP = 2048  # ACT call free size (4 PSUM banks)
N_MCHUNK = B // BT        # 16
N_GROUP = NSH // GROUP    # 4
JPG = GROUP // NB         # 4 matmuls per group

F32 = mybir.dt.float32
BF16 = mybir.dt.bfloat16

_PROGRAM_CACHE: dict[float, object] = {}
LAST_RESULTS = None  # BassKernelResults of the most recent run (for profiling)


def _build_program(s: float):
    AF = mybir.ActivationFunctionType
    ALU = mybir.AluOpType
    AX = mybir.AxisListType

    nc = bacc.Bacc(
        "TRN2",
        target_bir_lowering=False,
        debug=False,
        enable_asserts=False,
        num_devices=N_CORES,
    )
    svsT_d = nc.dram_tensor("svsT", [D, NSH], BF16, kind="ExternalInput").ap()
    xT_d = nc.dram_tensor("xT", [D, B], F32, kind="ExternalInput").ap()
    partial_d = nc.dram_tensor("partial", [B], F32, kind="ExternalOutput").ap()
    xrow_d = nc.dram_tensor("xrow", [B], F32, kind="ExternalOutput").ap()

    # constant folded into the per-query row (uses the GLOBAL N)
    cconst = float(-np.log(N_TOTAL) + (D / 2.0) * np.log(s / np.pi))

    with tile.TileContext(nc) as tc, ExitStack() as ctx:
        aug = ctx.enter_context(tc.tile_pool(name="aug", bufs=1))
        pp = ctx.enter_context(tc.tile_pool(name="psum", bufs=2, space="PSUM"))
        sp = ctx.enter_context(tc.tile_pool(name="scr", bufs=2))
        misc = ctx.enter_context(tc.tile_pool(name="misc", bufs=1))
        rowp = ctx.enter_context(tc.tile_pool(name="rowp", bufs=2))

        svst_aug = aug.tile([D + 1, NSH], BF16)
        xt_aug = aug.tile([D + 1, B], BF16)
        sq = misc.tile([D, NSH], BF16)       # svs^T squared elementwise
        xts = misc.tile([D, B], F32)         # raw X^T
        sqx = misc.tile([D, B], BF16)        # X^T squared elementwise
        negcol = misc.tile([D, 1], BF16)     # column of -s (partition reducer)
        accall = misc.tile([BT, N_MCHUNK * N_GROUP], F32)
        outp = misc.tile([BT, N_MCHUNK], F32)
        xrow_sb = misc.tile([1, B], F32)
        dum_i = misc.tile([1, 1], F32)
        dum_o = misc.tile([1, 1], F32)

        nc.vector.memset(negcol[:, :], 1.0)

        # ---- X-side prep on the otherwise-idle GPSIMD ----
        for k in range(2):
            c0 = k * (B // 2)
            c1 = c0 + B // 2
            nc.sync.dma_start(out=xts[:, c0:c1], in_=xT_d[:, c0:c1])
        nc.vector.tensor_scalar_mul(xt_aug[0:D, :], xts[:, :], 2.0 * s)
        nc.vector.memset(xt_aug[D : D + 1, :], 1.0)
        nc.vector.tensor_mul(sqx[:, :], xts[:, :], xts[:, :])

        # ---- y2-row prep, all groups up front (PE/DVE idle at start) ----
        for k in range(8):
            c0 = k * (NSH // 8)
            c1 = c0 + NSH // 8
            nc.sync.dma_start(out=svst_aug[0:D, c0:c1], in_=svsT_d[:, c0:c1])
        for g in range(N_GROUP):
            gc0 = g * GROUP
            nc.vector.tensor_mul(
                sq[:, gc0 : gc0 + GROUP],
                svst_aug[0:D, gc0 : gc0 + GROUP],
                svst_aug[0:D, gc0 : gc0 + GROUP],
            )
            psy = pp.tile([BT, GROUP], F32, tag="mm")
            for j in range(JPG):
                c0 = gc0 + j * NB
                nc.tensor.matmul(
                    psy[0:1, j * NB : (j + 1) * NB],
                    lhsT=negcol[:, :],
                    rhs=sq[:, c0 : c0 + NB],
                    start=True,
                    stop=True,
                )
            yrow = rowp.tile([1, GROUP], BF16)
            nc.vector.tensor_scalar_mul(yrow[0:1, :], psy[0:1, :], -s)
            # move row from partition 0 to partition 64 (SBUF->SBUF DMA)
            nc.sync.dma_start(
                out=svst_aug[D : D + 1, gc0 : gc0 + GROUP], in_=yrow[0:1, :]
            )

        # ---- main loop: matmul -> exp -> reduce ----
        for m in range(N_MCHUNK):
            for g in range(N_GROUP):
                idx = m * N_GROUP + g
                gc0 = g * GROUP
                ps = pp.tile([BT, GROUP], F32, tag="mm")
                for j in range(JPG):
                    col = gc0 + j * NB
                    nc.tensor.matmul(
                        ps[:, j * NB : (j + 1) * NB],
                        lhsT=xt_aug[:, m * BT : (m + 1) * BT],
                        rhs=svst_aug[:, col : col + NB],
                        start=True,
                        stop=True,
                    )
                scr = sp.tile([BT, GROUP], BF16)
                nc.scalar.activation(scr[:, :], ps[:, :], AF.Exp)
                # reduction via tensor_scalar bypass-mult + accum_out: unlike
                # tensor_reduce, InstTensorScalarPtr runs in the 4x_2p DVE
                # perf mode (all-SBUF, bf16) -> 0.25 cyc/elem
                nc.vector.tensor_reduce(
                    accall[:, idx : idx + 1], scr[:, :], axis=AX.X, op=ALU.add
                )

        # ---- fold the per-group partials and store ----
        acc3 = accall[:, :].rearrange("p (m g) -> p m g", g=N_GROUP)
        nc.vector.tensor_reduce(outp[:, :], acc3, axis=AX.X, op=ALU.add)
        nc.sync.dma_start(
            out=partial_d.rearrange("(m p) -> p m", p=BT), in_=outp[:, :]
        )

        # ---- xrow = -s*||x||^2 + cconst (tail; PE/DVE have slack) ----
        psx = pp.tile([BT, GROUP], F32, tag="mm")
        for j in range(JPG):
            nc.tensor.matmul(
                psx[0:1, j * NB : (j + 1) * NB],
                lhsT=negcol[:, :],
                rhs=sqx[:, j * NB : (j + 1) * NB],
                start=True,
                stop=True,
            )
        nc.vector.tensor_scalar(
            xrow_sb[0:1, :], psx[0:1, 0:B], -s, cconst, op0=ALU.mult, op1=ALU.add
        )
        nc.sync.dma_start(out=xrow_d[:], in_=xrow_sb[0:1, :])

    nc.compile()
    return nc


def _get_program(s: float):
    key = float(s)
    if key not in _PROGRAM_CACHE:
        _PROGRAM_CACHE[key] = _build_program(key)
    return _PROGRAM_CACHE[key]


def kernel(X, svs, scale, _trace=False):
    global LAST_RESULTS
    Xnp = np.asarray(X, dtype=np.float32)
    svs_np = np.asarray(svs, dtype=np.float32)
    s = float(np.asarray(scale))
    assert Xnp.shape == (B, D) and svs_np.shape == (N_TOTAL, D)

    nc = _get_program(s)

    xT = np.ascontiguousarray(Xnp.T)  # [64, 2048] f32, replicated
    in_maps = []
    for c in range(N_CORES):
        shard = svs_np[c * NSH : (c + 1) * NSH]
        svsT_c = np.ascontiguousarray(shard.T).astype(ml_dtypes.bfloat16)
        in_maps.append({"svsT": svsT_c, "xT": xT})

    res = run_bass_kernel_spmd(nc, in_maps, list(range(N_CORES)), trace=_trace)
    LAST_RESULTS = res

    partials = np.stack(
        [np.asarray(r["partial"], dtype=np.float64) for r in res.results]
    )  # [8, 2048]
    xrow = np.asarray(res.results[0]["xrow"], dtype=np.float64)
    out = np.log(partials.sum(axis=0)) + xrow
    return out.astype(np.float32)
